# revision 1
# baseline (speedup 1.0000x reference)
"""Causal self-attention (b=2, t=2048, d=1024, h=16) on 8 trn2 NeuronCores.

Sharding: core c handles batch c//4 and the 4 heads 4*(c%4)..4*(c%4)+3
(data parallel over batch x tensor parallel over heads). Each core
computes x @ w_qkv for its head-slice, causal attention for its heads,
and a partial out-projection  y_heads @ w_out[head_rows]; the host sums
the 4 partial outputs per batch (the tensor-parallel all-reduce).

Per-core kernel layout (everything f32r = fp32 storage, reduced-precision
matmul mode, 1 cyc/row on the PE at N>=256):
  xT [d, t] built via bf16 hi/lo split + DMA-transpose + DVE add
  qT, kT [dh, t] per head-pair (128 partitions = 2 heads x 64)
  v natural [t, dh] with a fused ones column -> PV matmul emits both
  y_unnorm and the softmax denominator; scores are O(5) so exp needs
  no max-subtraction. S is computed transposed ([j, i]) so softmax
  renormalization is a reciprocal + K=1 broadcast matmul.
"""

import numpy as np
import ml_dtypes

import concourse.bacc as bacc
import concourse.mybir as mybir
import concourse.tile as tile
from concourse.bass_utils import run_bass_kernel_spmd

F32 = mybir.dt.float32
F32R = mybir.dt.float32r
BF16 = mybir.dt.bfloat16

T = 2048            # sequence length
D = 1024            # model dim
DH = 64             # head dim
HPC = 4             # heads per core
NCORES = 8
NTT = T // 128      # 16 t-tiles of 128
NDC = D // 128      # 8 d-chunks of 128
NIB = T // 512      # 4 i-blocks of 512
JPB = 512 // 128    # j-chunks per i-block


def _build():
    nc = bacc.Bacc("TRN2", target_bir_lowering=False, debug=False)

    XHI = nc.dram_tensor("XHI", [T, D], BF16, kind="ExternalInput")
    XLO = nc.dram_tensor("XLO", [T, D], BF16, kind="ExternalInput")
    WQ = nc.dram_tensor("WQ", [D, 256], F32, kind="ExternalInput")
    WK = nc.dram_tensor("WK", [D, 256], F32, kind="ExternalInput")
    WV = nc.dram_tensor("WV", [D, 256], F32, kind="ExternalInput")
    WO = nc.dram_tensor("WO", [256, D], F32, kind="ExternalInput")
    TRI = nc.dram_tensor("TRI", [128, 128], F32, kind="ExternalInput")
    ONESC = nc.dram_tensor("ONESC", [128, NTT, HPC, 1], F32, kind="ExternalInput")
    ONES1 = nc.dram_tensor("ONES1", [1, 64], F32, kind="ExternalInput")
    OUT = nc.dram_tensor("OUT", [T, D], F32, kind="ExternalOutput")

    with tile.TileContext(nc) as tc:
        with tc.tile_pool(name="persist", bufs=1) as pp:
            qt = [pp.tile([128, T], F32R, tag=f"qt{p}", name=f"qt{p}") for p in range(2)]
            kt = [pp.tile([128, T], F32R, tag=f"kt{p}", name=f"kt{p}") for p in range(2)]
            vones = pp.tile([128, NTT, HPC, DH + 1], F32R, tag="vones")
            ypair = [pp.tile([128, T], F32R, tag=f"yp{p}", name=f"yp{p}") for p in range(2)]
            tri = pp.tile([128, 128], F32R, tag="tri")
            ones1 = pp.tile([1, 64], F32R, tag="ones1")
            wo_sb = pp.tile([128, 2, D], F32R, tag="wo")

            with tc.tile_pool(name="ldstage", bufs=2) as lds:
                for dst_ap, src_ap in (
                        (tri[:], TRI[:]),
                        (ones1[:], ONES1[:]),
                        (vones[:, :, :, DH:DH + 1], ONESC[:]),
                        (wo_sb[:], WO[:].rearrange("(c p) e -> p c e", p=128)),
                ):
                    st = lds.tile(list(dst_ap.shape), F32, tag="ldst")
                    nc.sync.dma_start(st[:], src_ap)
                    nc.vector.tensor_copy(dst_ap, st[:])

            # ---------------- phase A: xT + projections ----------------
            with tc.tile_pool(name="phA", bufs=1) as pa, \
                 tc.tile_pool(name="phAhl", bufs=2) as pahl, \
                 tc.tile_pool(name="psA", bufs=4, space="PSUM") as psa:
                xt = pa.tile([128, NDC, T], F32R, tag="xt")
                wq_sb = pa.tile([128, NDC, 256], F32R, tag="wq")
                wk_sb = pa.tile([128, NDC, 256], F32R, tag="wk")
                wv_sb = pa.tile([128, NDC, 256], F32R, tag="wv")
                with tc.tile_pool(name="wstage", bufs=2) as ws:
                    for w_dst, w_src in ((wq_sb, WQ), (wk_sb, WK), (wv_sb, WV)):
                        st = ws.tile([128, NDC, 256], F32, tag="wst")
                        nc.sync.dma_start(
                            st[:], w_src[:].rearrange("(c p) n -> p c n", p=128))
                        nc.vector.tensor_copy(w_dst[:], st[:])

                for dc in range(NDC):
                    xthi = pahl.tile([128, T], BF16, tag="xthi")
                    xtlo = pahl.tile([128, T], BF16, tag="xtlo")
                    nc.sync.dma_start(
                        xthi[:], XHI[:, dc * 128:(dc + 1) * 128], transpose=True)
                    nc.sync.dma_start(
                        xtlo[:], XLO[:, dc * 128:(dc + 1) * 128], transpose=True)
                    nc.vector.tensor_add(xt[:, dc, :], xthi[:], xtlo[:])

                # v projection: v[t, dh] for 4 heads, natural layout
                for ti in range(NTT):
                    vp = psa.tile([128, 256], F32, tag="vp")
                    for dc in range(NDC):
                        nc.tensor.matmul(
                            vp[:], xt[:, dc, ti * 128:(ti + 1) * 128],
                            wv_sb[:, dc, :],
                            start=(dc == 0), stop=(dc == NDC - 1))
                    nc.vector.tensor_copy(
                        vones[:, ti, :, 0:DH],
                        vp[:].rearrange("p (h d) -> p h d", h=HPC))

                # q/k projections, transposed layout, head-pairs of 128
                for dst, w_sb in ((qt, wq_sb), (kt, wk_sb)):
                    for pi in range(2):
                        for ib in range(NIB):
                            qp = psa.tile([128, 512], F32, tag="qkp")
                            for dc in range(NDC):
                                nc.tensor.matmul(
                                    qp[:],
                                    w_sb[:, dc, pi * 128:(pi + 1) * 128],
                                    xt[:, dc, ib * 512:(ib + 1) * 512],
                                    start=(dc == 0), stop=(dc == NDC - 1))
                            nc.vector.tensor_copy(
                                dst[pi][:, ib * 512:(ib + 1) * 512], qp[:])

            # ---------------- phase B: causal attention ----------------
            with tc.tile_pool(name="phB", bufs=1) as pb, \
                 tc.tile_pool(name="phBpt", bufs=3) as pbpt, \
                 tc.tile_pool(name="phBn", bufs=2) as pbn, \
                 tc.tile_pool(name="psBst", bufs=2, space="PSUM") as psbst, \
                 tc.tile_pool(name="psBy", bufs=1, space="PSUM") as psby, \
                 tc.tile_pool(name="psBbc", bufs=2, space="PSUM") as psbbc:
                for pi in range(2):
                    for ib in range(NIB):
                        jlast = JPB * ib + JPB - 1
                        ya = psby.tile([65, 512], F32, tag="ya")
                        yb = psby.tile([65, 512], F32, tag="yb")
                        for jc in range(jlast + 1):
                            off = 128 * (jc - JPB * ib) if jc >= JPB * ib else 0
                            n = 512 - off
                            sta = psbst.tile([128, 512], F32, tag="sta")
                            stb = psbst.tile([128, 512], F32, tag="stb")
                            pta = pbpt.tile([128, 512], F32R, tag="pta")
                            ptb = pbpt.tile([128, 512], F32R, tag="ptb")
                            js = slice(jc * 128, (jc + 1) * 128)
                            isl = slice(ib * 512 + off, (ib + 1) * 512)
                            nc.tensor.matmul(
                                sta[:, off:512], kt[pi][0:64, js],
                                qt[pi][0:64, isl], start=True, stop=True)
                            nc.tensor.matmul(
                                stb[:, off:512], kt[pi][64:128, js],
                                qt[pi][64:128, isl], start=True, stop=True,
                                tile_position=(64, 0))
                            nc.scalar.activation(
                                pta[:, off:512], sta[:, off:512],
                                mybir.ActivationFunctionType.Exp, scale=0.125)
                            nc.scalar.activation(
                                ptb[:, off:512], stb[:, off:512],
                                mybir.ActivationFunctionType.Exp, scale=0.125)
                            if jc >= JPB * ib:  # diagonal chunk: mask triangle
                                nc.vector.tensor_mul(
                                    pta[:, off:off + 128],
                                    pta[:, off:off + 128], tri[:])
                                nc.vector.tensor_mul(
                                    ptb[:, off:off + 128],
                                    ptb[:, off:off + 128], tri[:])
                            nc.tensor.matmul(
                                ya[0:65, off:512], vones[:, jc, 2 * pi, :],
                                pta[:, off:512],
                                start=(jc == 0), stop=(jc == jlast))
                            nc.tensor.matmul(
                                yb[0:65, off:512], vones[:, jc, 2 * pi + 1, :],
                                ptb[:, off:512],
                                start=(jc == 0), stop=(jc == jlast))
                        # renormalize: y /= denom (row 64)
                        ibs = slice(ib * 512, (ib + 1) * 512)
                        for head, yps, rows in ((0, ya, slice(0, 64)),
                                                (1, yb, slice(64, 128))):
                            rec = pbn.tile([1, 512], F32R, tag="rec")
                            bc = psbbc.tile([64, 512], F32, tag="bc")
                            bcs = pbn.tile([64, 512], F32R, tag="bcs")
                            with nc.allow_low_precision(
                                    reason="f32r reciprocal of softmax denom"):
                                nc.vector.reciprocal(rec[:], yps[64:65, :])
                            nc.tensor.matmul(
                                bc[:], ones1[:], rec[:], start=True, stop=True)
                            nc.vector.tensor_copy(bcs[:], bc[:])
                            nc.vector.tensor_mul(
                                ypair[pi][rows, ibs], yps[0:64, :], bcs[:])

            # ---------------- phase C: out-projection ----------------
            with tc.tile_pool(name="phC", bufs=2) as pc_, \
                 tc.tile_pool(name="psC", bufs=4, space="PSUM") as psc:
                for ti in range(NTT):
                    ost = pc_.tile([128, D], F32, tag="ost")
                    for eh in range(2):
                        op = psc.tile([128, 512], F32, tag="op")
                        for pi in range(2):
                            nc.tensor.matmul(
                                op[:], ypair[pi][:, ti * 128:(ti + 1) * 128],
                                wo_sb[:, pi, eh * 512:(eh + 1) * 512],
                                start=(pi == 0), stop=(pi == 1))
                        nc.vector.tensor_copy(
                            ost[:, eh * 512:(eh + 1) * 512], op[:])
                    nc.sync.dma_start(OUT[ti * 128:(ti + 1) * 128, :], ost[:])

    nc.compile()
    return nc


_NC = None


def build_in_maps(x, w_qkv, w_out):
    x = np.asarray(x, np.float32)
    w_qkv = np.asarray(w_qkv, np.float32)
    w_out = np.asarray(w_out, np.float32)

    tri = np.triu(np.ones((128, 128), np.float32))          # tri[j,i]=1 iff j<=i
    onesc = np.ones((128, NTT, HPC, 1), np.float32)
    ones1 = np.ones((1, 64), np.float32)

    in_maps = []
    for c in range(NCORES):
        b, g = divmod(c, 4)
        xb = x[b]
        xhi = xb.astype(ml_dtypes.bfloat16)
        xlo = (xb - xhi.astype(np.float32)).astype(ml_dtypes.bfloat16)
        cs = slice(g * 256, (g + 1) * 256)
        in_maps.append({
            "XHI": xhi, "XLO": xlo,
            "WQ": np.ascontiguousarray(w_qkv[:, cs]),
            "WK": np.ascontiguousarray(w_qkv[:, 1024:2048][:, cs]),
            "WV": np.ascontiguousarray(w_qkv[:, 2048:3072][:, cs]),
            "WO": np.ascontiguousarray(w_out[g * 256:(g + 1) * 256, :]),
            "TRI": tri, "ONESC": onesc, "ONES1": ones1,
        })
    return in_maps


def kernel(x, w_qkv, w_out):
    global _NC
    if _NC is None:
        _NC = _build()

    in_maps = build_in_maps(x, w_qkv, w_out)
    res = run_bass_kernel_spmd(_NC, in_maps, core_ids=list(range(NCORES)))
    outs = [res.results[c]["OUT"] for c in range(NCORES)]
    y = np.stack([outs[0] + outs[1] + outs[2] + outs[3],
                  outs[4] + outs[5] + outs[6] + outs[7]], axis=0)
    return y.astype(np.float32)



# revision 12
# speedup vs baseline: 1.5621x; 1.5621x over previous
"""Causal self-attention (b=2, t=2048, d=1024, h=16) on 8 trn2 NeuronCores.

Sharding: core c handles batch c//4 and the 4 heads 4*(c%4)..4*(c%4)+3
(data parallel over batch x tensor parallel over heads). Each core
computes x @ w_qkv for its head-slice, causal attention for its heads,
and a partial out-projection y_heads @ w_out[head_rows]; the host sums
the 4 partial outputs per batch (the tensor-parallel all-reduce).

v2: bf16 end-to-end (rel err ~6e-3, gate is 2e-2).
  - all inputs loaded via DMA-transpose (2-byte dtype) split across the
    sync + scalar HWDGE queues; no hi/lo split, no staging copies.
  - causal mask folded into the S PSUM accumulation as a matmul with a
    constant -BIG upper-triangle (maskT @ I), so exp(scale*(S+mask))=0
    above the diagonal -- no DVE masking pass.
  - one fused exp per j-chunk over both heads ([128, 2, 512] PSUM tile).
  - softmax denominator via fused ones-column in V (row 64 of the PV
    accumulator); renorm = ACT reciprocal + gpsimd partition_broadcast
    + DVE multiply.
  - ib-outer loop with the out-projection interleaved one half-block
    behind attention, so the PE stream stays dense (HAM warm).
"""

import numpy as np
import ml_dtypes

import concourse.bacc as bacc
import concourse.mybir as mybir
import concourse.tile as tile
from concourse.bass_utils import run_bass_kernel_spmd

F32 = mybir.dt.float32
BF16 = mybir.dt.bfloat16

T = 2048            # sequence length
D = 1024            # model dim
DH = 64             # head dim
HPC = 4             # heads per core
NCORES = 8
NTT = T // 128      # 16 t-tiles of 128
NDC = D // 128      # 8 d-chunks of 128
NIB = T // 512      # 4 i-blocks of 512
JPB = 512 // 128    # j-chunks per i-block
BIG = 30000.0


def _build():
    nc = bacc.Bacc("TRN2", target_bir_lowering=False, debug=False)

    XB = nc.dram_tensor("XB", [T, D], BF16, kind="ExternalInput")
    WQKVT = nc.dram_tensor("WQKVT", [768, D], BF16, kind="ExternalInput")
    WOT = nc.dram_tensor("WOT", [D, 256], BF16, kind="ExternalInput")
    MASKT = nc.dram_tensor("MASKT", [128, 128], BF16, kind="ExternalInput")
    IDENT = nc.dram_tensor("IDENT", [128, 128], BF16, kind="ExternalInput")
    OUT = nc.dram_tensor("OUT", [T, D], BF16, kind="ExternalOutput")

    EXP = mybir.ActivationFunctionType.Exp
    RECIP = mybir.ActivationFunctionType.Reciprocal

    with tile.TileContext(nc) as tc:
        with tc.tile_pool(name="persist", bufs=1) as pp, \
             tc.tile_pool(name="pt", bufs=4) as ppt, \
             tc.tile_pool(name="prec", bufs=2) as prec, \
             tc.tile_pool(name="pbc", bufs=2) as pbc, \
             tc.tile_pool(name="post", bufs=3) as post, \
             tc.tile_pool(name="psS", bufs=2, space="PSUM") as psS, \
             tc.tile_pool(name="psY", bufs=1, space="PSUM") as psY, \
             tc.tile_pool(name="psO", bufs=2, space="PSUM") as psO:

            xt = pp.tile([128, NDC, T], BF16, tag="xt")
            wsb = pp.tile([128, NDC, 768], BF16, tag="wsb")
            wo_sb = pp.tile([128, 2, D], BF16, tag="wo")
            qt = [pp.tile([128, T], BF16, tag=f"qt{p}", name=f"qt{p}")
                  for p in range(2)]
            kt = [pp.tile([128, T], BF16, tag=f"kt{p}", name=f"kt{p}")
                  for p in range(2)]
            vones = pp.tile([128, NTT, HPC, DH + 1], BF16, tag="vones")
            ypair = [pp.tile([128, T], BF16, tag=f"yp{p}", name=f"yp{p}")
                     for p in range(2)]
            maskt = pp.tile([128, 128], BF16, tag="maskt")
            ident = pp.tile([128, 128], BF16, tag="ident")

            # ---- input DMAs ----
            # small consts first (normal xbar mode), then all transposes.
            nc.sync.dma_start(maskt[:], MASKT[:])
            nc.sync.dma_start(ident[:], IDENT[:])
            nc.gpsimd.memset(vones[:, :, :, DH:DH + 1], 1.0)
            # interleave x / w chunks across the two HWDGE queues so the
            # dc=k chunk pair (xt, wsb) lands as early as possible.
            for dc in range(NDC):
                nc.sync.dma_start(
                    wsb[:, dc, :], WQKVT[:, dc * 128:(dc + 1) * 128],
                    transpose=True)
                nc.sync.dma_start(
                    xt[:, dc, :], XB[:, dc * 128:(dc + 1) * 128],
                    transpose=True)
            for pi in range(2):
                nc.sync.dma_start(
                    wo_sb[:, pi, :], WOT[:, pi * 128:(pi + 1) * 128],
                    transpose=True)

            # ---- phase A helpers ----
            def emit_qk(pi):
                for base, dst in ((0, qt[pi]), (256, kt[pi])):
                    for ib in range(NIB):
                        qp = psS.tile([128, 512], F32, tag="stab")
                        for dc in range(NDC):
                            nc.tensor.matmul(
                                qp[:],
                                wsb[:, dc, base + pi * 128:base + (pi + 1) * 128],
                                xt[:, dc, ib * 512:(ib + 1) * 512],
                                start=(dc == 0), stop=(dc == NDC - 1))
                        nc.vector.tensor_copy(
                            dst[:, ib * 512:(ib + 1) * 512], qp[:])

            def emit_v(t0, t1):
                for ti in range(t0, t1):
                    vp = psS.tile([128, 256], F32, tag="stab")
                    for dc in range(NDC):
                        nc.tensor.matmul(
                            vp[:], xt[:, dc, ti * 128:(ti + 1) * 128],
                            wsb[:, dc, 512:768],
                            start=(dc == 0), stop=(dc == NDC - 1))
                    nc.vector.tensor_copy(
                        vones[:, ti, :, 0:DH],
                        vp[:].rearrange("p (h d) -> p h d", h=HPC))

            # ---- phase B block: attention for (ib, pi) ----
            def emit_attn(ib, pi):
                yab = psY.tile([65, 2, 512], F32, tag="yab")
                njc = JPB * ib + JPB
                for jc in range(njc):
                    diag = jc >= JPB * ib
                    off = 128 * (jc - JPB * ib) if diag else 0
                    stab = psS.tile([128, 2, 512], F32, tag="stab")
                    ptab = ppt.tile([128, 2, 512], BF16, tag="ptab")
                    js = slice(jc * 128, (jc + 1) * 128)
                    isl = slice(ib * 512 + off, (ib + 1) * 512)
                    nc.tensor.matmul(
                        stab[:, 0, off:512], kt[pi][0:64, js],
                        qt[pi][0:64, isl], start=True, stop=not diag,
                        skip_group_check=True)
                    nc.tensor.matmul(
                        stab[:, 1, off:512], kt[pi][64:128, js],
                        qt[pi][64:128, isl], start=True, stop=not diag,
                        tile_position=(64, 0), skip_group_check=True)
                    if diag:
                        nc.tensor.matmul(
                            stab[:, 0, off:off + 128], maskt[:], ident[:],
                            start=False, stop=True, skip_group_check=True)
                        nc.tensor.matmul(
                            stab[:, 1, off:off + 128], maskt[:], ident[:],
                            start=False, stop=True, skip_group_check=True)
                    nc.scalar.activation(
                        ptab[:, :, off:512], stab[:, :, off:512],
                        EXP, scale=0.125)
                    nc.tensor.matmul(
                        yab[0:65, 0, off:512], vones[:, jc, 2 * pi, :],
                        ptab[:, 0, off:512],
                        start=(jc == 0), stop=(jc == njc - 1),
                        skip_group_check=True)
                    nc.tensor.matmul(
                        yab[0:65, 1, off:512], vones[:, jc, 2 * pi + 1, :],
                        ptab[:, 1, off:512],
                        start=(jc == 0), stop=(jc == njc - 1),
                        skip_group_check=True)
                # renorm: y /= denom (row 64), both heads at once
                den_sb = prec.tile([1, 2, 512], F32, tag="den")
                rec = prec.tile([1, 2, 512], F32, tag="rec")
                bcs = pbc.tile([64, 2, 512], F32, tag="bcs")
                nc.vector.tensor_copy(den_sb[:], yab[64:65, :, :])
                nc.vector.reciprocal_approx_fast(rec[:], den_sb[:])
                nc.gpsimd.partition_broadcast(bcs[:], rec[:])
                ibs = slice(ib * 512, (ib + 1) * 512)
                nc.vector.tensor_mul(
                    ypair[pi][0:64, ibs], yab[0:64, 0, :], bcs[0:64, 0, :])
                nc.vector.tensor_mul(
                    ypair[pi][64:128, ibs], yab[0:64, 1, :], bcs[0:64, 1, :])

            # ---- phase C block: out-projection for i-block ib ----
            def emit_outproj(ib):
                for ti in range(JPB * ib, JPB * ib + JPB):
                    ost = post.tile([128, D], BF16, tag="ost")
                    for eh in range(2):
                        op = psO.tile([128, 512], F32, tag="op")
                        nc.tensor.matmul(
                            op[:], ypair[0][:, ti * 128:(ti + 1) * 128],
                            wo_sb[:, 0, eh * 512:(eh + 1) * 512],
                            start=True, stop=False)
                        nc.tensor.matmul(
                            op[:], ypair[1][:, ti * 128:(ti + 1) * 128],
                            wo_sb[:, 1, eh * 512:(eh + 1) * 512],
                            start=False, stop=True)
                        nc.vector.tensor_copy(
                            ost[:, eh * 512:(eh + 1) * 512], op[:])
                    nc.scalar.dma_start(
                        OUT[ti * 128:(ti + 1) * 128, :], ost[:])

            # ---- emission schedule ----
            # A(pi0) -> B(0,0) -> A(pi1) -> B(0,1) -> v tail per ib;
            # out-proj for ib lands one half-block behind its renorm so
            # the PE queue never waits on the renorm chain.
            emit_qk(0)
            emit_v(0, 4)
            emit_attn(0, 0)
            emit_qk(1)
            emit_attn(0, 1)
            emit_v(4, 8)
            emit_attn(1, 0)
            emit_attn(1, 1)
            emit_outproj(0)
            emit_v(8, 12)
            emit_attn(2, 0)
            emit_attn(2, 1)
            emit_outproj(1)
            emit_v(12, 16)
            emit_attn(3, 0)
            emit_attn(3, 1)
            emit_outproj(2)
            emit_outproj(3)

    nc.compile()
    return nc


_NC = None


def build_in_maps(x, w_qkv, w_out):
    x = np.asarray(x, np.float32)
    w_qkv = np.asarray(w_qkv, np.float32)
    w_out = np.asarray(w_out, np.float32)

    idx = np.arange(128)
    maskt = np.where(idx[None, :] > idx[:, None], -BIG, 0.0).astype(
        ml_dtypes.bfloat16)                       # maskt[i,j] = -BIG iff j>i
    identm = np.eye(128, dtype=ml_dtypes.bfloat16)

    in_maps = []
    for c in range(NCORES):
        b, g = divmod(c, 4)
        cs = slice(g * 256, (g + 1) * 256)
        wq = w_qkv[:, 0 * 1024:1 * 1024][:, cs]
        wk = w_qkv[:, 1 * 1024:2 * 1024][:, cs]
        wv = w_qkv[:, 2 * 1024:3 * 1024][:, cs]
        wqkvt = np.ascontiguousarray(
            np.concatenate([wq, wk, wv], axis=1).T).astype(ml_dtypes.bfloat16)
        wot = np.ascontiguousarray(
            w_out[g * 256:(g + 1) * 256, :].T).astype(ml_dtypes.bfloat16)
        in_maps.append({
            "XB": x[b].astype(ml_dtypes.bfloat16),
            "WQKVT": wqkvt,
            "WOT": wot,
            "MASKT": maskt,
            "IDENT": identm,
        })
    return in_maps


def kernel(x, w_qkv, w_out):
    global _NC
    if _NC is None:
        _NC = _build()

    in_maps = build_in_maps(x, w_qkv, w_out)
    res = run_bass_kernel_spmd(_NC, in_maps, core_ids=list(range(NCORES)))
    outs = [res.results[c]["OUT"].astype(np.float32) for c in range(NCORES)]
    y = np.stack([outs[0] + outs[1] + outs[2] + outs[3],
                  outs[4] + outs[5] + outs[6] + outs[7]], axis=0)
    return y.astype(np.float32)
